# revision 17
# baseline (speedup 1.0000x reference)
"""Trainium2 Bass kernel for the CreditDecomposer problem.

Math (per token t, with D=512, P=4, H=64):
    q = x @ Wq + bq ; k = prim @ Wk + bk ; v = prim @ Wv + bv
    scores = (q @ k.T) / sqrt(D)          -> folds to  x @ Keff + bs
    attn   = softmax(scores)              -> (exp(s) * exp(bs)) / sum(...)
    weighted = attn @ v
    h = relu([x, weighted] @ W1 + b1)     -> relu(x @ W1a + attn @ V1p)
        where V1p = v @ W1b + 1*b1  (attn rows sum to 1)
    credit = sigmoid(h @ W2 + b2) = 0.5*tanh(0.5*(h @ W2 + b2)) + 0.5

Device kernel computes, per 128-token tile (x supplied pre-transposed):
    z[:, 0:68] = xT_chunks.T @ [Keff | W1a]   (4 accumulating matmuls, bf16)
    e = exp(z[:, 0:4]) ; ew = e * exp(bs) ; se = sum(ew) ; attn = ew / se
    attnT = PE-transpose(attn) ; z[:, 4:68] += attnT.T @ V1p
    credit = 0.5*tanh(0.5*(sum(relu(z[:, 4:68]) * W2) + b2)) + 0.5

Sharding: pure data-parallel, batch split 8 ways (8 batches/core).
Host pre/post: fold weights, transpose+cast x per core, un-transpose outputs.
"""

import sys

sys.path.insert(0, "/opt/trn_rl_repo")

from contextlib import ExitStack

import ml_dtypes
import numpy as np

import concourse.bass as bass
import concourse.tile as tile
from concourse import bacc, mybir
from concourse.masks import make_identity

F32 = mybir.dt.float32
BF16 = mybir.dt.bfloat16
AF = mybir.ActivationFunctionType
OP = mybir.AluOpType

B, S, D, P, H = 64, 4096, 512, 4, 64
NCORES = 8
T_CORE = (B // NCORES) * S  # 32768 tokens per core
NOUT = P + H  # 68 psum columns: [scores | mlp-hidden]
SLAB = 1024  # tokens per input DMA
BLK = 512  # tokens per attn-output block (4 tiles)


SUP = 4096  # tokens per attn-output superblock (8 blocks)


def build_program(T, b2_val):
    """Build the Bass program for T tokens (per core). Returns nc."""
    assert T % SUP == 0
    NT = T // 128  # 128-token tiles
    NB = T // BLK  # 4-tile blocks
    NS = T // SLAB  # input slabs
    NSUP = T // SUP

    nc = bacc.Bacc("TRN2", target_bir_lowering=False, debug=False)

    xt_d = nc.dram_tensor("xt", [128, 4, T], BF16, kind="ExternalInput")
    wc_d = nc.dram_tensor("wc", [128, 4, NOUT], BF16, kind="ExternalInput")
    v1_d = nc.dram_tensor("v1", [4, P, H], F32, kind="ExternalInput")
    wexp_d = nc.dram_tensor("wexp", [1, P], F32, kind="ExternalInput")
    w2_d = nc.dram_tensor("w2", [1, H], F32, kind="ExternalInput")
    # attnT[sup, p, k, n]; token t = sup*4096 + k*128 + n  (k = tile in super)
    attn_d = nc.dram_tensor(
        "attnT", [NSUP, P, SUP // 128, 128], F32, kind="ExternalOutput"
    )
    cred_d = nc.dram_tensor("credit", [128, NT], F32, kind="ExternalOutput")

    with tile.TileContext(nc) as tc, ExitStack() as ctx:
        const = ctx.enter_context(tc.tile_pool(name="const", bufs=1))
        slab_pool = ctx.enter_context(tc.tile_pool(name="slab", bufs=4))
        zp_pool = ctx.enter_context(tc.tile_pool(name="zp", bufs=4, space="PSUM"))
        at_pool = ctx.enter_context(tc.tile_pool(name="atp", bufs=4, space="PSUM"))
        small = ctx.enter_context(tc.tile_pool(name="small", bufs=12))
        scrap_pool = ctx.enter_context(tc.tile_pool(name="scrap", bufs=4))
        stage_pool = ctx.enter_context(tc.tile_pool(name="stage", bufs=2))

        # --- constants ---
        wc_s = const.tile([128, 4, NOUT], BF16)
        nc.sync.dma_start(wc_s[:], wc_d[:])
        v1_s = const.tile([P, H], F32)
        nc.sync.dma_start(v1_s[:], v1_d[0])
        wexp_s = const.tile([128, P], F32)
        nc.sync.dma_start(wexp_s[:], wexp_d[0:1, :].to_broadcast((128, P)))
        w2_s = const.tile([128, H], F32)
        nc.sync.dma_start(w2_s[:], w2_d[0:1, :].to_broadcast((128, H)))
        ident = const.tile([128, 128], F32)
        make_identity(nc, ident[:])
        b2_s = const.tile([128, 1], F32)
        nc.vector.memset(b2_s[:], 0.5 * float(b2_val))
        cred_s = const.tile([128, NT], F32)

        blocks_per_sup = SUP // BLK  # 8
        stage = None
        for s in range(NS):
            slab = slab_pool.tile([128, 4, SLAB], BF16)
            nc.sync.dma_start(slab[:], xt_d[:, :, s * SLAB : (s + 1) * SLAB])
            for bi in range(SLAB // BLK):
                blk = s * (SLAB // BLK) + bi
                sup, bb = divmod(blk, blocks_per_sup)
                if bb == 0:
                    stage = stage_pool.tile([P, SUP // 128 * 128], F32)
                # 4 tiles of this block share one PSUM bank: cols [68j : 68j+68]
                zp = zp_pool.tile([128, 4 * NOUT], F32)
                # one accumulation group for the whole bank: start=True clears
                # has_written BANK-wide, so only the block's first matmul starts
                for j in range(4):
                    t0 = bi * BLK + j * 128
                    for c in range(4):
                        nc.tensor.matmul(
                            zp[:, j * NOUT : j * NOUT + NOUT],
                            lhsT=slab[:, c, t0 : t0 + 128],
                            rhs=wc_s[:, c, :],
                            start=(j == 0 and c == 0),
                            stop=(j == 3 and c == 3),
                            skip_group_check=True,
                        )
                # batched exp over the block's 4 score groups (strided AP)
                e_blk = small.tile([128, 4, P], F32)
                zp_v = zp[:].rearrange("p (j n) -> p j n", j=4)
                nc.scalar.activation(e_blk[:], zp_v[:, :, 0:P], AF.Exp)
                cr_blk = small.tile([128, 4], F32)
                for j in range(4):
                    ew = small.tile([128, P], F32)
                    se = small.tile([128, 1], F32)
                    nc.vector.scalar_tensor_tensor(
                        ew[:], e_blk[:, j, :], 1.0, wexp_s[:], OP.mult, OP.mult,
                        accum_out=se[:],
                    )
                    rinv = small.tile([128, 1], F32)
                    nc.vector.reciprocal(rinv[:], se[:])
                    attn = small.tile([128, P], F32)
                    nc.vector.tensor_scalar(attn[:], ew[:], rinv[:], None, OP.mult)
                    atp = at_pool.tile([P, 128], F32)
                    nc.tensor.transpose(atp[:], attn[:], ident[:])
                    k = bb * 4 + j  # tile within super
                    dst = stage[:, k * 128 : (k + 1) * 128]
                    if j % 2 == 0:
                        nc.vector.tensor_copy(dst, atp[:])
                    else:
                        nc.scalar.copy(dst, atp[:])
                for j in range(4):
                    k = bb * 4 + j
                    nc.tensor.matmul(
                        zp[:, j * NOUT + P : (j + 1) * NOUT],
                        lhsT=stage[:, k * 128 : (k + 1) * 128],
                        rhs=v1_s[:],
                        start=False,
                        stop=True,
                        skip_group_check=True,
                    )
                for j in range(4):
                    scrap = scrap_pool.tile([128, H], F32)
                    nc.vector.scalar_tensor_tensor(
                        scrap[:], zp[:, j * NOUT + P : (j + 1) * NOUT], 0.0, w2_s[:],
                        OP.max, OP.mult, accum_out=cr_blk[:, j : j + 1],
                    )
                ct = small.tile([128, 4], F32)
                nc.scalar.activation(ct[:], cr_blk[:], AF.Tanh, bias=b2_s[:], scale=0.5)
                nc.vector.tensor_scalar(
                    cred_s[:, blk * 4 : blk * 4 + 4], ct[:], 0.5, 0.5, OP.mult, OP.add
                )
                if bb == blocks_per_sup - 1:
                    nc.scalar.dma_start(attn_d[sup], stage[:])
        nc.sync.dma_start(cred_d[:], cred_s[:])

    nc.compile()
    return nc


def _fold_weights(primitive_sequence, emb, Wq, bq, Wk, bk, Wv, bv, W1, b1, W2, b2):
    """Host-side algebraic folds, in float64 for accuracy."""
    idx = np.asarray(primitive_sequence).astype(np.int64)
    prim = np.asarray(emb, np.float64)[idx]  # (P, D)
    k = prim @ np.asarray(Wk, np.float64) + np.asarray(bk, np.float64)
    v = prim @ np.asarray(Wv, np.float64) + np.asarray(bv, np.float64)
    inv = 1.0 / np.sqrt(D)
    Keff = (np.asarray(Wq, np.float64) @ k.T) * inv  # (D, P)
    bs = (np.asarray(bq, np.float64) @ k.T) * inv  # (P,)
    W1a = np.asarray(W1, np.float64)[:D]  # (D, H)
    W1b = np.asarray(W1, np.float64)[D:]  # (D, H)
    V1p = v @ W1b + np.asarray(b1, np.float64)[None, :]  # (P, H)
    Wc = np.concatenate([Keff, W1a], axis=1)  # (D, NOUT)
    return Wc, bs, V1p, np.asarray(W2, np.float64), float(np.asarray(b2).reshape(-1)[0])


_CACHE = {}


def _get_program(T, b2_val):
    key = (T, round(float(b2_val), 12))
    if key not in _CACHE:
        _CACHE[key] = build_program(T, b2_val)
    return _CACHE[key]


def _pack_x_core(x_core):
    """(T, D) f32 -> (128, 4, T) bf16 with xt[p, c, t] = x[t, 128c+p]."""
    T = x_core.shape[0]
    xt = x_core.reshape(T, 4, 128).transpose(2, 1, 0)  # (128, 4, T) view
    return np.ascontiguousarray(xt).astype(ml_dtypes.bfloat16)


def _make_in_maps(trajectory_features, Wc, bs, V1p, W2, T):
    wc_host = np.ascontiguousarray(
        Wc.reshape(4, 128, NOUT).transpose(1, 0, 2)
    ).astype(ml_dtypes.bfloat16)  # (128, 4, NOUT), wc[p, c, n] = Wc[128c+p, n]
    v1_host = np.ascontiguousarray(
        np.broadcast_to(V1p.astype(np.float32), (4, P, H))
    )
    wexp_host = np.exp(bs).astype(np.float32).reshape(1, P)
    w2_host = W2.astype(np.float32).reshape(1, H)

    x = np.asarray(trajectory_features, np.float32).reshape(B * S, D)
    per_core = T
    in_maps = []
    for c in range(NCORES):
        xc = x[c * per_core : (c + 1) * per_core]
        in_maps.append(
            {
                "xt": _pack_x_core(xc),
                "wc": wc_host,
                "v1": v1_host,
                "wexp": wexp_host,
                "w2": w2_host,
            }
        )
    return in_maps


LAST_RESULTS = None  # stash for test harness (exec time etc.)


def kernel(
    trajectory_features,
    primitive_sequence,
    emb,
    Wq,
    bq,
    Wk,
    bk,
    Wv,
    bv,
    W1,
    b1,
    W2,
    b2,
    _trace=False,
):
    global LAST_RESULTS
    from concourse.bass_utils import run_bass_kernel_spmd

    Wc, bs, V1p, W2f, b2_val = _fold_weights(
        primitive_sequence, emb, Wq, bq, Wk, bk, Wv, bv, W1, b1, W2, b2
    )
    nc = _get_program(T_CORE, b2_val)
    in_maps = _make_in_maps(trajectory_features, Wc, bs, V1p, W2f, T_CORE)
    res = run_bass_kernel_spmd(
        nc, in_maps, list(range(NCORES)), trace=_trace
    )
    LAST_RESULTS = res

    credit = np.empty((NCORES, T_CORE), np.float32)
    attn = np.empty((NCORES, T_CORE, P), np.float32)
    for c in range(NCORES):
        out = res.results[c]
        # credit dram is (128, NT): token t = tile*128 + p
        credit[c] = out["credit"].T.reshape(-1)
        # attnT dram is (NSUP, P, 32, 128): t = sup*4096 + k*128 + n
        attn[c] = out["attnT"].transpose(0, 2, 3, 1).reshape(T_CORE, P)
    credit = credit.reshape(B, S)
    attn = attn.reshape(B, S, P)
    return credit, attn
